# revision 34
# baseline (speedup 1.0000x reference)
"""Trainium2 Bass kernel for MetaDynamics potential evaluation.

out[p] = sum_h hgt[h] * exp(-0.5 * sum_d (cen[h,d]-col[p,d])^2 / wdt[h,d]^2)
H=16384 hills, P=4096 points, D=8 CVs; hills sharded over 8 cores (2048 each).

Quadratic form -> rank-17 inner product  e'[h,p] = F[p] . W[h]:
  W = [cen*c, -c/2, -a/2],  F = [col, col^2, 1],  c = 1/wdt^2,
  a = sum cen^2 c - 2 ln hgt  (so e' = ln hgt - e/2 and out = sum_h exp e').

Main matmuls run in fp8-e4m3 DoubleRow perf mode (2 cols/cycle): both factors
are split into 5 fp8 levels and all cross products with level_i+level_j <= 4
are stacked -> 15 blocks x 17 rows = 255 logical K (128 partitions x 2).
Measured end-to-end L2 error of this split: 1.5e-3 (gate 2e-2).

Per-core the 2048 hills split across two pointwise pipelines, sized so the
Scalar and Vector engines finish together (both ~85% busy):

* ACT path (1024 hills): psum [128 pts, 1024 hills] ping-pong (4 banks);
  ScalarE Exp with accum_out gives the per-point partial sums for free.
* DVE bit path (1024 hills, 4 group-pair waves): psum [128 hills, 512 pts]
  (3 rotating banks); VectorE computes y = A8*e' + B8 (A8 = 4/ln2) and
  casts to uint8 with round-to-nearest + saturation: the u8 bit pattern IS
  fp8-e5m2(exp(e') * 2^13) (Schraudolph; the 2^13 pre-scale keeps hills
  down to e' = -20 above e5m2's subnormal floor, and negative y saturates
  to +0.0). Pairs of 128-hill groups pack into [128, 2, 512] u8 tiles so
  one DoubleRow matmul against a partition-selector contracts 256 hills
  into psum row c of a single shared bank. sigma8 = -0.225 centers the
  2-bit-mantissa sawtooth (L2 ~3e-3 in sim, 1.6e-3 measured end to end).

Matmuls are emitted group-major inside each wave so consecutive PE
instructions share stationary weights (a weight swap costs ~100-170ns of
pipeline drain), and ACT p-tiles interleave every other chunk to keep
ScalarE fed. The host sums the 8 cores' two partial outputs (bits row
divided by 2^13).

Measured: 63.3us vs the 84.2us ACT-only baseline; L2 err 1.6e-3 (gate 2e-2).
"""

import numpy as np
import ml_dtypes

import concourse.bacc as bacc
import concourse.mybir as mybir
import concourse.tile as tile
from concourse import bass_utils



H, P, D = 16384, 4096, 8
NCORES = 8
HL = H // NCORES            # hills per core
HA = 1024                   # ACT-path hills per core
HB = HL - HA                # bit-path hills per core (1024 = 8 groups)
PT = 128                    # points per ACT p-tile
NPT = P // PT               # 32
NLEV = 5                    # fp8 split levels
BLOCKS = [(i, j) for i in range(NLEV) for j in range(NLEV) if i + j <= 4]
KROWS = len(BLOCKS) * 17    # 255
DVE_T = 512                 # DVE psum tile points (1 bank, 3 bufs)
NG = HB // 128              # bit-path hill groups
NCH = P // DVE_T            # point chunks per group
SUM_LAG = 2                 # dve-steps between tensor_scalar and its sums

LN2 = float(np.log(2.0))
# e5m2 Schraudolph constants: u8 bits = A8*e' + B8 bitcast as fp8e5 give
# exp(e')*2^13 (the 2^13 pre-scale keeps e' down to -20 above e5m2's
# subnormal floor); sigma8 centers the 2-bit-mantissa sawtooth (simmed
# L2 ~3e-3 at -0.225 on this problem's inputs).
A8 = 4.0 / LN2
SIGMA8 = -0.225
B8 = 4.0 * (15 + 13) + SIGMA8
BIT_SCALE = float(2.0 ** 13)

BF16 = mybir.dt.bfloat16
F32 = mybir.dt.float32
U8 = mybir.dt.uint8
FP8 = mybir.dt.float8e4
FP8E5 = mybir.dt.float8e5

_NC_CACHE = None


def _build_nc():
    nc = bacc.Bacc(
        "TRN2",
        target_bir_lowering=False,
        debug=False,
        enable_asserts=False,
        num_devices=NCORES,
    )
    ft = nc.dram_tensor("ft", [128, 2, P], FP8, kind="ExternalInput").ap()
    wt = nc.dram_tensor("wt", [128, 2, HL], FP8, kind="ExternalInput").ap()
    # sel[k, c, i, m] = 1 if m == c: DoubleRow ones-contraction that routes
    # chunk c's column sums to psum partition c (matmul out base partition
    # must be 0). m-dim padded to 16 so the middle AP stride is 16B.
    sel = nc.dram_tensor("sel", [128, 8, 2, 16], FP8E5, kind="ExternalInput").ap()
    acta = nc.dram_tensor("acta", [PT, NPT], F32, kind="ExternalOutput").ap()
    bits = nc.dram_tensor("bits", [8, 512], F32, kind="ExternalOutput").ap()

    with tile.TileContext(nc) as tc:
        with (
            tc.tile_pool(name="const", bufs=1) as cpool,
            tc.tile_pool(name="bitsb", bufs=NCH + 1) as bpool,
            tc.tile_pool(name="actp", bufs=2, space="PSUM") as apool,
            tc.tile_pool(name="dvep", bufs=3, space="PSUM") as dpool,
            tc.tile_pool(name="sump", bufs=1, space="PSUM") as spool,
        ):
            ftt = cpool.tile([128, 2, P], FP8)
            wtt = cpool.tile([128, 2, HL], FP8)
            selt = cpool.tile([128, 8, 2, 16], FP8E5)
            acc = cpool.tile([PT, NPT], F32)
            dummy = cpool.tile([128, 1], F32)
            sums_sb = cpool.tile([8, 512], F32)

            # Input DMAs spread over three DGE queues so the transfers issue
            # in parallel; the scalar queue's DMA goes out before the exp
            # table load occupies its sequencer.
            nc.sync.dma_start(selt[:], sel)
            nc.sync.dma_start(wtt[:, :, 0:HA], wt[:, :, 0:HA])
            nc.scalar.dma_start(ftt[:, :, 0:1024], ft[:, :, 0:1024])
            nc.gpsimd.dma_start(wtt[:, :, HA:HL], wt[:, :, HA:HL])
            nc.sync.dma_start(ftt[:, :, 1024:P], ft[:, :, 1024:P])

            nc.vector.memset(dummy[:], 0.0)
            # Load the exp table set while input DMAs run.
            nc.scalar.activation(dummy[:], dummy[:],
                                 mybir.ActivationFunctionType.Exp, scale=1.0)

            sums = spool.tile([128, 512], F32)
            DR = mybir.MatmulPerfMode.DoubleRow
            NW = NG // 2          # group-pair waves
            bts = {}

            acts = iter(range(NPT))

            def act_step():
                i = next(acts, None)
                if i is None:
                    return False
                pt = apool.tile([PT, HA], F32, name="actt")
                lhs = ftt[:, :, i * PT:(i + 1) * PT]
                for c0 in range(0, HA, 512):
                    nc.tensor.matmul(pt[:, c0:c0 + 512], lhsT=lhs,
                                     rhs=wtt[:, :, c0:c0 + 512],
                                     start=True, stop=True, perf_mode=DR)
                nc.scalar.activation(pt[:], pt[:],
                                     mybir.ActivationFunctionType.Exp,
                                     scale=1.0, accum_out=acc[:, i:i + 1])
                return True

            def dve_step(w, half, c):
                gs = HA + (2 * w + half) * 128
                pt = dpool.tile([128, DVE_T], F32, name="dvet")
                nc.tensor.matmul(pt[:], lhsT=wtt[:, :, gs:gs + 128],
                                 rhs=ftt[:, :, c * DVE_T:(c + 1) * DVE_T],
                                 start=True, stop=True, perf_mode=DR)
                if half == 0:
                    bts[c] = bpool.tile([128, 2, DVE_T], U8, name="bits")
                nc.vector.tensor_scalar(bts[c][:, half, :], pt[:], A8, B8,
                                        mybir.AluOpType.mult,
                                        mybir.AluOpType.add)

            def pair_sum(w, c):
                nc.tensor.matmul(sums[0:8, :],
                                 lhsT=selt[:, c, :, 0:8],
                                 rhs=bts[c].bitcast(FP8E5),
                                 start=(w == 0 and c == 0),
                                 stop=(w == NW - 1 and c == NCH - 1),
                                 perf_mode=DR, skip_group_check=True)

            # Wave schedule: per group-pair wave, each group's 8 chunk
            # matmuls run group-major (stationary weights shared), with ACT
            # p-tiles interleaved every other chunk; the pair's u8 bit tiles
            # are contracted 256 hills at a time at the wave tail.
            # Two ACT p-tiles first: their inputs arrive on the fast sync
            # queue, so the in-order PE has work before the gpsimd-issued
            # bit-path weight DMA lands.
            act_step()
            act_step()
            for w in range(NW):
                for c in range(NCH):
                    dve_step(w, 0, c)
                    if c % 2 == 0:
                        act_step()
                for c in range(NCH):
                    dve_step(w, 1, c)
                    # chunk c's pair is complete: contract it immediately so
                    # the sums hide under the remaining tensor_scalars
                    # instead of trailing the wave.
                    pair_sum(w, c)
                    if c % 2 == 1:
                        act_step()
            while act_step():
                pass

            nc.vector.tensor_copy(sums_sb[:], sums[0:8, :])
            nc.sync.dma_start(acta, acc[:])
            nc.sync.dma_start(bits, sums_sb[:])

    nc.compile()
    return nc


def _get_nc():
    global _NC_CACHE
    if _NC_CACHE is None:
        _NC_CACHE = _build_nc()
    return _NC_CACHE


def _split_fp8(x):
    parts = []
    r = x.astype(np.float64)
    for _ in range(NLEV):
        p = r.astype(ml_dtypes.float8_e4m3)
        parts.append(p)
        r = r - p.astype(np.float64)
    return parts


def _pack_rows(parts_17cols, level_of_block, n):
    """Stack the 15 17-row blocks into [128, 2, n] fp8 (row r -> (r%128, r//128))."""
    out = np.zeros((128, 2, n), dtype=ml_dtypes.float8_e4m3)
    r = 0
    for b in range(len(BLOCKS)):
        lev = level_of_block[b]
        blk = parts_17cols[lev]  # [n, 17]
        for t in range(17):
            out[r % 128, r // 128, :] = blk[:, t]
            r += 1
    return out


def _prepare_inputs(col, cen, wdt, hgt):
    col64 = col.astype(np.float64)
    cen64 = cen.astype(np.float64)
    wdt64 = wdt.astype(np.float64)
    hgt64 = np.maximum(hgt.astype(np.float64), 1e-300)

    c = 1.0 / (wdt64 * wdt64)
    a = np.sum(cen64 * cen64 * c, axis=1) - 2.0 * np.log(hgt64)
    W = np.concatenate([cen64 * c, -0.5 * c, (-0.5 * a)[:, None]], axis=1)  # [H,17]
    F = np.concatenate([col64, col64 * col64, np.ones((P, 1))], axis=1)     # [P,17]

    Fp = _split_fp8(F)
    Wp = _split_fp8(W)
    ft8 = _pack_rows(Fp, [b[0] for b in BLOCKS], P)
    wt8_full = _pack_rows(Wp, [b[1] for b in BLOCKS], H)
    wts = [np.ascontiguousarray(wt8_full[:, :, i * HL:(i + 1) * HL])
           for i in range(NCORES)]
    sel = np.zeros((128, 8, 2, 16), dtype=ml_dtypes.float8_e5m2)
    for cc in range(8):
        sel[:, cc, :, cc] = 1.0
    return np.ascontiguousarray(ft8), wts, sel


def run_on_hw(col, cen, wdt, hgt, trace=False):
    ft8, wts, sel = _prepare_inputs(col, cen, wdt, hgt)
    nc = _get_nc()
    in_maps = [{"ft": ft8, "wt": wts[i], "sel": sel} for i in range(NCORES)]
    res = bass_utils.run_bass_kernel_spmd(
        nc, in_maps, core_ids=list(range(NCORES)), trace=trace
    )
    total = np.zeros(P, dtype=np.float64)
    for r in res.results:
        total += r["acta"].T.reshape(P).astype(np.float64)
        total += r["bits"].reshape(P).astype(np.float64) / BIT_SCALE
    return total.astype(np.float32), res


def kernel(col, cen, wdt, hgt):
    out, _ = run_on_hw(col, cen, wdt, hgt, trace=False)
    return out
